# revision 7
# baseline (speedup 1.0000x reference)
"""MultiHeadLiftLayer Trainium2 kernel (dma_gather quad-table design).

reference:
    edge_signal = relu(x_0[src] @ W[:C] + x_0[tgt] @ W[C:])   # [E, 8]
    out = concat([edge_signal, x_1], axis=1)                   # [E, 72]

Strategy (8 NeuronCores, edges sharded; per core E_CORE = 78125 edges):
  Phase 1 (PE): project all nodes once: P[n, 0:8] = x_0[n] @ W[:C],
    P[n, 8:16] = x_0[n] @ W[C:], written to a DRAM quad-table
    qtab[12544, 64] f32 where row q = nodes 4q..4q+3 (16 f32 each). 256B rows
    satisfy dma_gather's elem-size constraint; idx = node>>2 < 12544 fits the
    int16 index requirement.
  Phase 2 (SWDGE + DVE): per 4096-edge chunk one dma_gather (transpose=False,
    single_packet=False — the only flavor that works on this HW) fetches the
    src quad and tgt quad per edge (8192 idx) round-robined over the 4 SWDGE
    queues (a single queue serializes at ~8ns/idx; 4 queues give ~2.5ns/idx).
    DVE selects the node-within-quad band by host-provided predication masks
    (base copy + 3 copy_predicated per endpoint), adds src+tgt, ACT applies
    relu into the p-major staging tile, where x_1 is DMA'd alongside and the
    result stored as one contiguous run per partition.
"""
import sys

sys.path.insert(0, "/opt/trn_rl_repo")

import numpy as np
import concourse.bass as bass
import concourse.tile as tile
from concourse import bacc, mybir
from concourse.bass_utils import run_bass_kernel_spmd

NUM_NODES = 50000
NPAD = 50176            # 392 * 128
NQUAD = NPAD // 4       # 12544
IN_CH0 = 128
HEADS = 8
NUM_EDGES = 625000
IN_CH1 = 64
OUT_CH = HEADS + IN_CH1  # 72

N_CORES = 8
E_CORE = NUM_EDGES // N_CORES  # 78125
CH = 4096                # edges per main chunk
NB = CH // 128           # 32 p-major cols per endpoint
N_MAIN = 19              # 19 * 4096 = 77824
TAIL = E_CORE - N_MAIN * CH          # 301
TAIL_SLOTS = 512                     # padded slots for the tail chunk
TAIL_NB = TAIL_SLOTS // 128          # 4
NCH = N_MAIN + 1
STRIP = 2048             # phase-1 node strip
BLK_GRP = 8              # node blocks per staging flush

_cache = {}


def _build_program():
    if "nc" in _cache:
        return _cache["nc"]
    nc = bacc.Bacc("TRN2", target_bir_lowering=False, debug=False,
                   num_devices=N_CORES, dynamic_dma_scratch_size=65536,
                   num_swdge_queues=4)
    f32, f16, i16 = mybir.dt.float32, mybir.dt.float16, mybir.dt.int16
    u8 = mybir.dt.uint8

    x0t = nc.dram_tensor("x0t", [IN_CH0, NPAD], f16, kind="ExternalInput").ap()
    wcat = nc.dram_tensor("wcat", [IN_CH0, 16], f32, kind="ExternalInput").ap()
    x1 = nc.dram_tensor("x1", [E_CORE, IN_CH1], f32, kind="ExternalInput").ap()
    # per chunk: 2*CH indices (src quads then tgt quads), wrapped in 16
    qidx = nc.dram_tensor("qidx", [NCH, 128, 2 * CH // 16], i16,
                          kind="ExternalInput").ap()
    # per chunk: [128, 2 endpoints * 3 classes, NB, 8] u8 predication masks
    emask = nc.dram_tensor("emask", [NCH, 128, 6, NB, 8], u8,
                           kind="ExternalInput").ap()
    out = nc.dram_tensor("out", [E_CORE, OUT_CH], f32, kind="ExternalOutput").ap()

    with tile.TileContext(nc) as tc:
        with tc.tile_pool(name="qt", bufs=1, space="DRAM") as qt_pool, \
             tc.tile_pool(name="const", bufs=1) as const_pool:
            qtab = qt_pool.tile([NQUAD, 64], f32)
            w32 = const_pool.tile([128, 16], f32)
            nc.sync.dma_start(w32[:], wcat[:])
            wc16 = const_pool.tile([128, 16], f16)
            nc.vector.tensor_copy(wc16[:], w32[:])

            # ---- phase 1: build the projected quad table ----
            with tc.tile_pool(name="p1s", bufs=2) as strip_pool, \
                 tc.tile_pool(name="p1g", bufs=2) as stg_pool, \
                 tc.tile_pool(name="p1ps", bufs=4, space="PSUM") as p1ps:
                n_strips = NPAD // STRIP                  # 24 full strips of 2048
                rem_nodes = NPAD - n_strips * STRIP       # 1024
                for t in range(n_strips + (1 if rem_nodes else 0)):
                    n0 = t * STRIP
                    ns = min(STRIP, NPAD - n0)
                    st = strip_pool.tile([128, STRIP], f16, tag="st")
                    nc.sync.dma_start(st[:, :ns], x0t[:, n0:n0 + ns])
                    nblk = ns // 128
                    for g0 in range(0, nblk, BLK_GRP):
                        gn = min(BLK_GRP, nblk - g0)
                        stg = stg_pool.tile([128, BLK_GRP, 16], f32, tag="stg")
                        for k in range(gn):
                            ps = p1ps.tile([128, 16], f32)
                            blk = st[:, (g0 + k) * 128:(g0 + k + 1) * 128]
                            nc.tensor.matmul(ps[:], lhsT=blk, rhs=wc16[:],
                                             start=True, stop=True)
                            nc.scalar.copy(stg[:, k, :], ps[:])
                        q0 = (n0 + g0 * 128) // 4
                        dst = qtab[q0:q0 + gn * 32].rearrange(
                            "(b q) (j h) -> (q j) b h", b=gn, j=4)
                        nc.sync.dma_start(dst, stg[:, :gn, :])

            # ---- phase 2: gather / select / emit ----
            with tc.tile_pool(name="io", bufs=2) as io_pool, \
                 tc.tile_pool(name="gs", bufs=4) as gs_pool, \
                 tc.tile_pool(name="mega", bufs=3) as mega_pool:
                for i in range(NCH):
                    main = i < N_MAIN
                    nb = NB if main else TAIL_NB
                    nidx = 2 * 128 * nb
                    e0 = i * CH

                    it = io_pool.tile([128, 2 * CH // 16], i16, tag="it")
                    nc.sync.dma_start(it[:, :nidx // 16],
                                      qidx[i, :, :nidx // 16])
                    mk = io_pool.tile([128, 6, NB, 8], u8, tag="mk")
                    nc.sync.dma_start(mk[:, :, :nb, :], emask[i, :, :, :nb, :])

                    gs = gs_pool.tile([128, 2 * NB, 64], f32, tag="gs")
                    nc.gpsimd.dma_gather(
                        out_ap=gs[:, :2 * nb, :], in_ap=qtab[:],
                        idxs_ap=it[:, :nidx // 16],
                        num_idxs=nidx, num_idxs_reg=nidx, elem_size=64,
                        transpose=False, single_packet=False,
                        queue_num=i % 4)

                    mega = mega_pool.tile([128, NB, OUT_CH], f32, tag="mega")
                    asrc = io_pool.tile([128, NB, 8], f32, tag="asrc")
                    gsrc = gs[:, 0:nb, :]
                    gtgt = gs[:, nb:2 * nb, :]
                    nc.vector.tensor_copy(asrc[:, :nb, :], gsrc[:, :, 0:8])
                    for j in (1, 2, 3):
                        nc.vector.copy_predicated(
                            asrc[:, :nb, :], mk[:, j - 1, :nb, :],
                            gsrc[:, :, 16 * j:16 * j + 8])
                    atgt = io_pool.tile([128, NB, 8], f32, tag="atgt")
                    nc.vector.tensor_copy(atgt[:, :nb, :], gtgt[:, :, 8:16])
                    for j in (1, 2, 3):
                        nc.vector.copy_predicated(
                            atgt[:, :nb, :], mk[:, 2 + j, :nb, :],
                            gtgt[:, :, 16 * j + 8:16 * j + 16])
                    nc.vector.scalar_tensor_tensor(
                        asrc[:, :nb, :], asrc[:, :nb, :], 1.0, atgt[:, :nb, :],
                        op0=mybir.AluOpType.mult, op1=mybir.AluOpType.add)
                    nc.scalar.activation(mega[:, :nb, 0:8], asrc[:, :nb, :],
                                         mybir.ActivationFunctionType.Relu)

                    if main:
                        v = slice(e0, e0 + CH)
                        nc.sync.dma_start(
                            mega[:, :, HEADS:],
                            x1[v].rearrange("(p s) c -> p s c", s=NB))
                        nc.scalar.dma_start(
                            out[v].rearrange("(p s) c -> p s c", s=NB),
                            mega[:])
                    else:
                        # tail: seg-major, 301 valid edges in 512 slots
                        full_seg = TAIL // 128            # 2
                        rem = TAIL - full_seg * 128       # 45
                        if full_seg:
                            v = slice(e0, e0 + full_seg * 128)
                            nc.sync.dma_start(
                                mega[:, :full_seg, HEADS:],
                                x1[v].rearrange("(s p) c -> p s c", p=128))
                            nc.scalar.dma_start(
                                out[v].rearrange("(s p) c -> p s c", p=128),
                                mega[:, :full_seg, :])
                        if rem:
                            v = slice(e0 + full_seg * 128, e0 + TAIL)
                            nc.sync.dma_start(mega[:rem, full_seg, HEADS:],
                                              x1[v])
                            nc.scalar.dma_start(out[v],
                                                mega[:rem, full_seg, :])

    nc.compile()
    _cache["nc"] = nc
    return nc


def _prep_inputs(x_0, adjacency_0, x_1, att_parameter):
    x0t = np.zeros((IN_CH0, NPAD), np.float16)
    x0t[:, :NUM_NODES] = np.asarray(x_0).T.astype(np.float16)
    wcat = np.empty((IN_CH0, 16), np.float32)
    wcat[:, 0:8] = att_parameter[:IN_CH0]
    wcat[:, 8:16] = att_parameter[IN_CH0:]

    src_all = np.asarray(adjacency_0[0]).astype(np.int64)
    tgt_all = np.asarray(adjacency_0[1]).astype(np.int64)
    x_1 = np.asarray(x_1, dtype=np.float32)

    in_maps = []
    for core in range(N_CORES):
        lo = core * E_CORE
        src = src_all[lo:lo + E_CORE]
        tgt = tgt_all[lo:lo + E_CORE]
        qidx = np.full((NCH, 128, 2 * CH // 16), -1, np.int16)
        emask = np.zeros((NCH, 128, 6, NB, 8), np.uint8)
        for i in range(NCH):
            main = i < N_MAIN
            nb = NB if main else TAIL_NB
            e0 = i * CH
            ne = CH if main else TAIL
            slots = 128 * nb
            # slot s -> edge (p-major for main: e = e0 + 32p + c with
            # s = 128c + p; seg-major for tail: e = e0 + s)
            s = np.arange(slots)
            p, c = s % 128, s // 128
            if main:
                eidx = e0 + nb * p + c
                valid = np.ones(slots, bool)
            else:
                eidx = e0 + s
                valid = s < ne
                eidx = np.where(valid, eidx, e0)  # pad slots gather real data
            sv = src[eidx]
            tv = tgt[eidx]
            # gather idx stream: positions 0..slots-1 = src quads,
            # slots..2*slots-1 = tgt quads; wrapped idx[i%16, i//16]
            vals = np.concatenate([sv >> 2, tv >> 2]).astype(np.int16)
            n = np.arange(2 * slots)
            qidx[i, n % 16, n // 16] = vals
            qidx[i, :, :2 * slots // 16] = np.tile(
                qidx[i, :16, :2 * slots // 16], (8, 1))
            # masks: endpoint 0 (src) classes 1..3 at [:, j-1], endpoint 1
            # (tgt) at [:, 2+j]; mask[p, ., c, :] = 1 where node%4 == j
            sq = (sv & 3).reshape(slots)
            tq = (tv & 3).reshape(slots)
            for j in (1, 2, 3):
                ms = ((sq == j) & valid).astype(np.uint8)
                mt = ((tq == j) & valid).astype(np.uint8)
                emask[i, p, j - 1, c, :] = ms[:, None]
                emask[i, p, 2 + j, c, :] = mt[:, None]
        in_maps.append({
            "x0t": x0t,
            "wcat": wcat,
            "x1": x_1[lo:lo + E_CORE],
            "qidx": qidx,
            "emask": emask,
        })
    return in_maps


def kernel(x_0, adjacency_0, x_1, att_parameter, _trace=False):
    x_0 = np.asarray(x_0, dtype=np.float32)
    adjacency_0 = np.asarray(adjacency_0)
    x_1 = np.asarray(x_1, dtype=np.float32)
    att_parameter = np.asarray(att_parameter, dtype=np.float32)
    nc = _build_program()
    in_maps = _prep_inputs(x_0, adjacency_0, x_1, att_parameter)
    res = run_bass_kernel_spmd(nc, in_maps, list(range(N_CORES)), trace=_trace)
    out = np.concatenate([res.results[i]["out"] for i in range(N_CORES)], axis=0)
    kernel.last_exec_time_ns = res.exec_time_ns
    return out


# revision 8
# speedup vs baseline: 1.3690x; 1.3690x over previous
"""MultiHeadLiftLayer Trainium2 kernel (dma_gather quad-table design).

reference:
    edge_signal = relu(x_0[src] @ W[:C] + x_0[tgt] @ W[C:])   # [E, 8]
    out = concat([edge_signal, x_1], axis=1)                   # [E, 72]

Strategy (8 NeuronCores, edges sharded; per core E_CORE = 78125 edges):
  Phase 1 (PE): project all nodes once: P[n, 0:8] = x_0[n] @ W[:C],
    P[n, 8:16] = x_0[n] @ W[C:], written to a DRAM quad-table
    qtab[12544, 64] f32 where row q = nodes 4q..4q+3 (16 f32 each). 256B rows
    satisfy dma_gather's elem-size constraint; idx = node>>2 < 12544 fits the
    int16 index requirement.
  Phase 2 (SWDGE + DVE): per 4096-edge chunk one dma_gather (transpose=False,
    single_packet=False — the only flavor that works on this HW) fetches the
    src quad and tgt quad per edge (8192 idx) round-robined over the 4 SWDGE
    queues (a single queue serializes at ~8ns/idx; 4 queues give ~2.5ns/idx).
    DVE selects the node-within-quad band by host-provided predication masks
    (base copy + 3 copy_predicated per endpoint), adds src+tgt, ACT applies
    relu into the p-major staging tile, where x_1 is DMA'd alongside and the
    result stored as one contiguous run per partition.
"""
import sys

sys.path.insert(0, "/opt/trn_rl_repo")

import numpy as np
import concourse.bass as bass
import concourse.tile as tile
from concourse import bacc, mybir
from concourse.bass_utils import run_bass_kernel_spmd

NUM_NODES = 50000
NPAD = 50176            # 392 * 128
NQUAD = NPAD // 4       # 12544
IN_CH0 = 128
HEADS = 8
NUM_EDGES = 625000
IN_CH1 = 64
OUT_CH = HEADS + IN_CH1  # 72

N_CORES = 8
E_CORE = NUM_EDGES // N_CORES  # 78125
CH = 2048                # edges per main chunk
NB = CH // 128           # 16 p-major cols per endpoint
N_MAIN = 38              # 38 * 2048 = 77824
TAIL = E_CORE - N_MAIN * CH          # 301
TAIL_SLOTS = 512                     # padded slots for the tail chunk
TAIL_NB = TAIL_SLOTS // 128          # 4
NCH = N_MAIN + 1
STRIP = 2048             # phase-1 node strip
BLK_GRP = 8              # node blocks per staging flush

_cache = {}


def _build_program():
    if "nc" in _cache:
        return _cache["nc"]
    nc = bacc.Bacc("TRN2", target_bir_lowering=False, debug=False,
                   num_devices=N_CORES, dynamic_dma_scratch_size=65536,
                   num_swdge_queues=4)
    f32, f16, i16 = mybir.dt.float32, mybir.dt.float16, mybir.dt.int16
    u8 = mybir.dt.uint8

    x0t = nc.dram_tensor("x0t", [IN_CH0, NPAD], f16, kind="ExternalInput").ap()
    wcat = nc.dram_tensor("wcat", [IN_CH0, 16], f32, kind="ExternalInput").ap()
    x1 = nc.dram_tensor("x1", [E_CORE, IN_CH1], f32, kind="ExternalInput").ap()
    # per chunk: 2*CH indices (src quads then tgt quads), wrapped in 16
    qidx = nc.dram_tensor("qidx", [NCH, 128, 2 * CH // 16], i16,
                          kind="ExternalInput").ap()
    # per chunk: [128, 2 endpoints * 3 classes, NB, 8] u8 predication masks
    emask = nc.dram_tensor("emask", [NCH, 128, 6, NB, 8], u8,
                           kind="ExternalInput").ap()
    out = nc.dram_tensor("out", [E_CORE, OUT_CH], f32, kind="ExternalOutput").ap()

    with tile.TileContext(nc) as tc:
        with tc.tile_pool(name="qt", bufs=1, space="DRAM") as qt_pool, \
             tc.tile_pool(name="const", bufs=1) as const_pool:
            qtab = qt_pool.tile([NQUAD, 64], f32)
            w32 = const_pool.tile([128, 16], f32)
            nc.sync.dma_start(w32[:], wcat[:])
            wc16 = const_pool.tile([128, 16], f16)
            nc.vector.tensor_copy(wc16[:], w32[:])

            # ---- phase 1: build the projected quad table ----
            with tc.tile_pool(name="p1s", bufs=2) as strip_pool, \
                 tc.tile_pool(name="p1g", bufs=2) as stg_pool, \
                 tc.tile_pool(name="p1ps", bufs=4, space="PSUM") as p1ps:
                n_strips = NPAD // STRIP                  # 24 full strips of 2048
                rem_nodes = NPAD - n_strips * STRIP       # 1024
                for t in range(n_strips + (1 if rem_nodes else 0)):
                    n0 = t * STRIP
                    ns = min(STRIP, NPAD - n0)
                    st = strip_pool.tile([128, STRIP], f16, tag="st")
                    nc.sync.dma_start(st[:, :ns], x0t[:, n0:n0 + ns])
                    nblk = ns // 128
                    for g0 in range(0, nblk, BLK_GRP):
                        gn = min(BLK_GRP, nblk - g0)
                        stg = stg_pool.tile([128, BLK_GRP, 16], f32, tag="stg")
                        for k in range(gn):
                            ps = p1ps.tile([128, 16], f32)
                            blk = st[:, (g0 + k) * 128:(g0 + k + 1) * 128]
                            nc.tensor.matmul(ps[:], lhsT=blk, rhs=wc16[:],
                                             start=True, stop=True)
                            nc.scalar.copy(stg[:, k, :], ps[:])
                        q0 = (n0 + g0 * 128) // 4
                        dst = qtab[q0:q0 + gn * 32].rearrange(
                            "(b q) (j h) -> (q j) b h", b=gn, j=4)
                        nc.sync.dma_start(dst, stg[:, :gn, :])

            # ---- phase 2: gather / select / emit ----
            with tc.tile_pool(name="io", bufs=3) as io_pool, \
                 tc.tile_pool(name="gs", bufs=6) as gs_pool, \
                 tc.tile_pool(name="mega", bufs=3) as mega_pool:
                for i in range(NCH):
                    main = i < N_MAIN
                    nb = NB if main else TAIL_NB
                    nidx = 2 * 128 * nb
                    e0 = i * CH

                    it = io_pool.tile([128, 2 * CH // 16], i16, tag="it")
                    nc.sync.dma_start(it[:, :nidx // 16],
                                      qidx[i, :, :nidx // 16])
                    mk = io_pool.tile([128, 6, NB, 8], u8, tag="mk")
                    nc.sync.dma_start(mk[:, :, :nb, :], emask[i, :, :, :nb, :])

                    gs = gs_pool.tile([128, 2 * NB, 64], f32, tag="gs")
                    nc.gpsimd.dma_gather(
                        out_ap=gs[:, :2 * nb, :], in_ap=qtab[:],
                        idxs_ap=it[:, :nidx // 16],
                        num_idxs=nidx, num_idxs_reg=nidx, elem_size=64,
                        transpose=False, single_packet=False,
                        queue_num=i % 4)

                    mega = mega_pool.tile([128, NB, OUT_CH], f32, tag="mega")
                    asrc = io_pool.tile([128, NB, 8], f32, tag="asrc")
                    gsrc = gs[:, 0:nb, :]
                    gtgt = gs[:, nb:2 * nb, :]
                    nc.scalar.copy(asrc[:, :nb, :], gsrc[:, :, 0:8])
                    for j in (1, 2, 3):
                        nc.vector.copy_predicated(
                            asrc[:, :nb, :], mk[:, j - 1, :nb, :],
                            gsrc[:, :, 16 * j:16 * j + 8])
                    atgt = io_pool.tile([128, NB, 8], f32, tag="atgt")
                    nc.scalar.copy(atgt[:, :nb, :], gtgt[:, :, 8:16])
                    for j in (1, 2, 3):
                        nc.vector.copy_predicated(
                            atgt[:, :nb, :], mk[:, 2 + j, :nb, :],
                            gtgt[:, :, 16 * j + 8:16 * j + 16])
                    nc.vector.scalar_tensor_tensor(
                        asrc[:, :nb, :], asrc[:, :nb, :], 1.0, atgt[:, :nb, :],
                        op0=mybir.AluOpType.mult, op1=mybir.AluOpType.add)
                    nc.scalar.activation(mega[:, :nb, 0:8], asrc[:, :nb, :],
                                         mybir.ActivationFunctionType.Relu)

                    if main:
                        v = slice(e0, e0 + CH)
                        nc.sync.dma_start(
                            mega[:, :, HEADS:],
                            x1[v].rearrange("(p s) c -> p s c", s=NB))
                        nc.scalar.dma_start(
                            out[v].rearrange("(p s) c -> p s c", s=NB),
                            mega[:])
                    else:
                        # tail: seg-major, 301 valid edges in 512 slots
                        full_seg = TAIL // 128            # 2
                        rem = TAIL - full_seg * 128       # 45
                        if full_seg:
                            v = slice(e0, e0 + full_seg * 128)
                            nc.sync.dma_start(
                                mega[:, :full_seg, HEADS:],
                                x1[v].rearrange("(s p) c -> p s c", p=128))
                            nc.scalar.dma_start(
                                out[v].rearrange("(s p) c -> p s c", p=128),
                                mega[:, :full_seg, :])
                        if rem:
                            v = slice(e0 + full_seg * 128, e0 + TAIL)
                            nc.sync.dma_start(mega[:rem, full_seg, HEADS:],
                                              x1[v])
                            nc.scalar.dma_start(out[v],
                                                mega[:rem, full_seg, :])

    nc.compile()
    _cache["nc"] = nc
    return nc


def _prep_inputs(x_0, adjacency_0, x_1, att_parameter):
    x0t = np.zeros((IN_CH0, NPAD), np.float16)
    x0t[:, :NUM_NODES] = np.asarray(x_0).T.astype(np.float16)
    wcat = np.empty((IN_CH0, 16), np.float32)
    wcat[:, 0:8] = att_parameter[:IN_CH0]
    wcat[:, 8:16] = att_parameter[IN_CH0:]

    src_all = np.asarray(adjacency_0[0]).astype(np.int64)
    tgt_all = np.asarray(adjacency_0[1]).astype(np.int64)
    x_1 = np.asarray(x_1, dtype=np.float32)

    in_maps = []
    for core in range(N_CORES):
        lo = core * E_CORE
        src = src_all[lo:lo + E_CORE]
        tgt = tgt_all[lo:lo + E_CORE]
        qidx = np.full((NCH, 128, 2 * CH // 16), -1, np.int16)
        emask = np.zeros((NCH, 128, 6, NB, 8), np.uint8)
        for i in range(NCH):
            main = i < N_MAIN
            nb = NB if main else TAIL_NB
            e0 = i * CH
            ne = CH if main else TAIL
            slots = 128 * nb
            # slot s -> edge (p-major for main: e = e0 + 32p + c with
            # s = 128c + p; seg-major for tail: e = e0 + s)
            s = np.arange(slots)
            p, c = s % 128, s // 128
            if main:
                eidx = e0 + nb * p + c
                valid = np.ones(slots, bool)
            else:
                eidx = e0 + s
                valid = s < ne
                eidx = np.where(valid, eidx, e0)  # pad slots gather real data
            sv = src[eidx]
            tv = tgt[eidx]
            # gather idx stream: positions 0..slots-1 = src quads,
            # slots..2*slots-1 = tgt quads; wrapped idx[i%16, i//16]
            vals = np.concatenate([sv >> 2, tv >> 2]).astype(np.int16)
            n = np.arange(2 * slots)
            qidx[i, n % 16, n // 16] = vals
            qidx[i, :, :2 * slots // 16] = np.tile(
                qidx[i, :16, :2 * slots // 16], (8, 1))
            # masks: endpoint 0 (src) classes 1..3 at [:, j-1], endpoint 1
            # (tgt) at [:, 2+j]; mask[p, ., c, :] = 1 where node%4 == j
            sq = (sv & 3).reshape(slots)
            tq = (tv & 3).reshape(slots)
            for j in (1, 2, 3):
                ms = ((sq == j) & valid).astype(np.uint8)
                mt = ((tq == j) & valid).astype(np.uint8)
                emask[i, p, j - 1, c, :] = ms[:, None]
                emask[i, p, 2 + j, c, :] = mt[:, None]
        in_maps.append({
            "x0t": x0t,
            "wcat": wcat,
            "x1": x_1[lo:lo + E_CORE],
            "qidx": qidx,
            "emask": emask,
        })
    return in_maps


def kernel(x_0, adjacency_0, x_1, att_parameter, _trace=False):
    x_0 = np.asarray(x_0, dtype=np.float32)
    adjacency_0 = np.asarray(adjacency_0)
    x_1 = np.asarray(x_1, dtype=np.float32)
    att_parameter = np.asarray(att_parameter, dtype=np.float32)
    nc = _build_program()
    in_maps = _prep_inputs(x_0, adjacency_0, x_1, att_parameter)
    res = run_bass_kernel_spmd(nc, in_maps, list(range(N_CORES)), trace=_trace)
    out = np.concatenate([res.results[i]["out"] for i in range(N_CORES)], axis=0)
    kernel.last_exec_time_ns = res.exec_time_ns
    return out
